# revision 4
# baseline (speedup 1.0000x reference)
"""Multi-head attention (B=2, S=2048, D=1024, H=16) on 8 TRN2 NeuronCores.

Sharding: zero-collective sequence-data-parallel.  Core c handles batch
b = c // 4 and query chunk qc = c % 4 (512 query rows).  Each core computes
full K/V for its batch (all 16 heads, redundantly within the 4-core batch
group), Q only for its 512-row chunk, attention in scores.T [s, q]
orientation (no on-chip transposes; softmax without max-subtraction --
scores here are ~N(0, 0.41) so exp is safe), and the output projection for
its chunk.  No cross-core communication.

Layout choices are made host-side when sharding: contraction operands are
passed pre-transposed ([in_dim, out_dim], contraction on partitions) and
pre-rounded to bf16; accumulation on device is fp32 (PSUM).  Softmax
denominators come for free from a ones-column appended to V (the PV matmul
then yields sum(exp) in its last output row); the reciprocal is broadcast
across partitions with a K=1 matmul against a ones vector in float32r.
"""

import sys

for _p in ("/opt/trn_rl_repo",):
    if _p not in sys.path:
        sys.path.insert(0, _p)

import numpy as np
import ml_dtypes

import bass_rust
import concourse.bass as bass
import concourse.mybir as mybir
import concourse.tile as tile
from concourse.vector_clock import ScopedClock, VectorClock

F32 = mybir.dt.float32
F32R = mybir.dt.float32r
BF16 = mybir.dt.bfloat16
AF = mybir.ActivationFunctionType

D = 1024
S = 2048
SQ = 512
H = 16
DK = 64
NT_D = D // 128
NT_S = S // 128
NT_Q = SQ // 128
N_CORES = 8

# ---------------------------------------------------------------------------
# Workarounds for this walrus build, which accepts at most ONE semaphore wait
# per instruction ('Too many sync wait commands' in setupSyncWait).  Tile
# attaches multiple waits freely; split them across same-engine nops, and
# emit the kernel-tail drain one waited-semaphore at a time.
# ---------------------------------------------------------------------------

_WAITS_PER_INST = 1


def _split_drain_and_barrier(self, tick_clock, wait_clock):
    gc = tick_clock.global_clock
    n = len(gc)
    procs = [i for i in range(n) if gc[i] > 0]
    for i in range(0, len(procs), _WAITS_PER_INST):
        group = procs[i : i + _WAITS_PER_INST]
        vec = [0] * n
        for p in group:
            vec[p] = gc[p]
        drain_inst = self.nc.sync.drain()
        wait_clock.add_sem_waits(drain_inst.ins, ScopedClock({None: VectorClock(vec)}))

    self.nc.all_engine_barrier()
    assert self.sems is not None
    popped = self.nc._tile_sem_poison_stack.pop()
    assert popped is self._sem_poison
    self.nc.clear_and_free_semaphores(list(self.sems.allocated().values()))
    self.nc.all_engine_barrier()


tile.TileContext._drain_and_barrier = _split_drain_and_barrier


def _split_sync_waits(nc, limit=_WAITS_PER_INST):
    for f in nc.m.functions:
        for bb in f.blocks:
            insts = list(bb.instructions)
            if not any(
                inst.sync_info and len(inst.sync_info.on_wait or []) > limit
                for inst in insts
            ):
                continue
            new_list = []
            for inst in insts:
                si = inst.sync_info
                waits = list(si.on_wait) if si and si.on_wait else []
                if len(waits) > limit:
                    extra, keep = waits[:-limit], waits[-limit:]
                    for j in range(0, len(extra), limit):
                        chunk = extra[j : j + limit]
                        nop = nc.engines[inst.engine].nop(nofuse=True).ins
                        cur = nc.cur_bb.bb
                        assert cur.instructions[-1].name == nop.name
                        cur.instructions.pop()
                        nop.sync_info = bass_rust.SyncInfo(on_wait=chunk, on_update=[])
                        new_list.append(nop)
                    si.on_wait = keep
                new_list.append(inst)
            bb.instructions[:] = new_list


# ---------------------------------------------------------------------------
# Kernel builder
# ---------------------------------------------------------------------------


def build_mha(reps=1):
    """reps > 1 wraps the body in an on-device For_i loop (timing variant)."""
    nc = bass.Bass()
    xT_d = nc.declare_dram_parameter("xT", [D, S], BF16, isOutput=False)
    xTq_d = nc.declare_dram_parameter("xTq", [D, SQ], BF16, isOutput=False)
    WqT_d = nc.declare_dram_parameter("WqT", [D, D], BF16, isOutput=False)
    WkT_d = nc.declare_dram_parameter("WkT", [D, D], BF16, isOutput=False)
    WvT_d = nc.declare_dram_parameter("WvT", [D, D], BF16, isOutput=False)
    WoT_d = nc.declare_dram_parameter("WoT", [D, D], BF16, isOutput=False)
    bqt_d = nc.declare_dram_parameter("bqt", [128, NT_D], F32, isOutput=False)
    bkt_d = nc.declare_dram_parameter("bkt", [128, NT_D], F32, isOutput=False)
    bvr_d = nc.declare_dram_parameter("bvr", [1, D], BF16, isOutput=False)
    bor_d = nc.declare_dram_parameter("bor", [1, D], BF16, isOutput=False)
    Y_d = nc.declare_dram_parameter("y", [SQ, D], F32, isOutput=True)

    with tile.TileContext(nc) as tc:
        with tc.tile_pool(name="persist", bufs=1) as persist:
            KT = persist.tile([128, NT_D, S], BF16, name="KT")
            QT = persist.tile([128, NT_D, SQ], BF16, name="QT")
            VS = persist.tile([128, NT_S, H, DK + 1], BF16, name="VS")
            AT = persist.tile([128, NT_D, SQ], BF16, name="AT")
            bqt = persist.tile([128, NT_D], F32, name="bqt")
            bkt = persist.tile([128, NT_D], F32, name="bkt")
            bvr = persist.tile([1, D], BF16, name="bvr")
            bor = persist.tile([1, D], BF16, name="bor")
            ones1 = persist.tile([1, 128], BF16, name="ones1")
            ones64r = persist.tile([1, DK], F32R, name="ones64r")
            ones64f = persist.tile([1, DK], F32, name="ones64f")

            nc.sync.dma_start(out=bqt[:], in_=bqt_d[:])
            nc.sync.dma_start(out=bkt[:], in_=bkt_d[:])
            nc.sync.dma_start(out=bvr[:], in_=bvr_d[:])
            nc.sync.dma_start(out=bor[:], in_=bor_d[:])
            nc.vector.memset(ones1[:], 1.0)
            nc.vector.memset(ones64f[:], 1.0)
            nc.vector.tensor_copy(out=ones64r[:], in_=ones64f[:])
            nc.vector.memset(VS[:, :, :, DK : DK + 1], 1.0)

            def phase1():
                with (
                    tc.tile_pool(name="ph1", bufs=1) as ph1,
                    tc.tile_pool(name="ps_kt", bufs=1, space="PSUM") as ps_kt,
                    tc.tile_pool(name="ps_mm", bufs=2, space="PSUM") as ps_mm,
                ):
                    xT = ph1.tile([128, NT_D, S], BF16, name="xT")
                    xTq = ph1.tile([128, NT_D, SQ], BF16, name="xTq")
                    WkT = ph1.tile([128, NT_D, D], BF16, name="WkT")
                    WqT = ph1.tile([128, NT_D, D], BF16, name="WqT")
                    WvT = ph1.tile([128, NT_D, D], BF16, name="WvT")
                    nc.sync.dma_start(
                        out=xT[:], in_=xT_d.rearrange("(c p) s -> p c s", p=128)
                    )
                    nc.sync.dma_start(
                        out=WkT[:], in_=WkT_d.rearrange("(c p) o -> p c o", p=128)
                    )
                    nc.sync.dma_start(
                        out=xTq[:], in_=xTq_d.rearrange("(c p) s -> p c s", p=128)
                    )
                    nc.sync.dma_start(
                        out=WqT[:], in_=WqT_d.rearrange("(c p) o -> p c o", p=128)
                    )
                    nc.sync.dma_start(
                        out=WvT[:], in_=WvT_d.rearrange("(c p) o -> p c o", p=128)
                    )

                    for ot in range(NT_D):
                        # K.T block [o-tile, all s] in a 4-bank psum tile
                        pk = ps_kt.tile([128, 4, 512], F32, name="pk")
                        for sc in range(4):
                            for dt in range(NT_D):
                                nc.tensor.matmul(
                                    pk[:, sc, :],
                                    WkT[:, dt, ot * 128 : (ot + 1) * 128],
                                    xT[:, dt, sc * 512 : (sc + 1) * 512],
                                    start=(dt == 0),
                                    stop=(dt == NT_D - 1),
                                )
                        nc.scalar.activation(
                            KT[:, ot, :],
                            pk[:, :, :].rearrange("p a b -> p (a b)"),
                            AF.Identity,
                            bias=bkt[:, ot : ot + 1],
                        )

                        # V for two s-tiles (fills PE while ACT evicts K.T)
                        for st in (2 * ot, 2 * ot + 1):
                            for oc in range(2):
                                pv = ps_mm.tile([128, 512], F32, name="mm")
                                for dt in range(NT_D):
                                    nc.tensor.matmul(
                                        pv[:],
                                        xT[:, dt, st * 128 : (st + 1) * 128],
                                        WvT[:, dt, oc * 512 : (oc + 1) * 512],
                                        start=(dt == 0),
                                        stop=False,
                                    )
                                nc.tensor.matmul(
                                    pv[:],
                                    ones1[:],
                                    bvr[:, oc * 512 : (oc + 1) * 512],
                                    start=False,
                                    stop=True,
                                )
                                nc.vector.tensor_copy(
                                    out=VS[:, st, oc * 8 : (oc + 1) * 8, 0:DK],
                                    in_=pv[:].rearrange("p (h d) -> p h d", d=DK),
                                )

                        # Q.T block
                        pq = ps_mm.tile([128, 512], F32, name="mm")
                        for dt in range(NT_D):
                            nc.tensor.matmul(
                                pq[:],
                                WqT[:, dt, ot * 128 : (ot + 1) * 128],
                                xTq[:, dt, :],
                                start=(dt == 0),
                                stop=(dt == NT_D - 1),
                            )
                        nc.scalar.activation(
                            QT[:, ot, :], pq[:], AF.Identity, bias=bqt[:, ot : ot + 1]
                        )

            def phase23():
                with (
                    tc.tile_pool(name="ph2", bufs=1) as ph2,
                    tc.tile_pool(name="es", bufs=2) as esp,
                    tc.tile_pool(name="small", bufs=2) as small,
                    tc.tile_pool(name="ps_qk", bufs=2, space="PSUM") as ps_qk,
                    tc.tile_pool(name="ps_pv", bufs=2, space="PSUM") as ps_pv,
                    tc.tile_pool(name="ps_rb", bufs=1, space="PSUM") as ps_rb,
                ):
                    WoT = ph2.tile([128, NT_D, D], BF16, name="WoT")
                    nc.sync.dma_start(
                        out=WoT[:], in_=WoT_d.rearrange("(c p) o -> p c o", p=128)
                    )

                    for h in range(H):
                        ot, po = h // 2, DK * (h % 2)
                        es = esp.tile([128, NT_S, 512], BF16, name="es")
                        for sg in range(NT_S // 2):
                            pqk = ps_qk.tile([128, 2, 512], F32, name="pqk")
                            for j in range(2):
                                st = 2 * sg + j
                                nc.tensor.matmul(
                                    pqk[:, j, :],
                                    KT[po : po + DK, ot, st * 128 : (st + 1) * 128],
                                    QT[po : po + DK, ot, :],
                                    start=True,
                                    stop=True,
                                )
                            nc.scalar.activation(
                                es[:, 2 * sg : 2 * sg + 2, :].rearrange(
                                    "p a b -> p (a b)"
                                ),
                                pqk[:].rearrange("p a b -> p (a b)"),
                                AF.Exp,
                                scale=0.125,
                            )
                        ppv = ps_pv.tile([DK + 1, 512], F32, name="ppv")
                        for st in range(NT_S):
                            nc.tensor.matmul(
                                ppv[:],
                                VS[:, st, h, :],
                                es[:, st, :],
                                start=(st == 0),
                                stop=(st == NT_S - 1),
                            )
                        r = small.tile([1, 512], F32R, name="r")
                        with nc.allow_low_precision(reason="f32r recip, 1.2e-4 rel"):
                            nc.vector.reciprocal(out=r[:], in_=ppv[DK : DK + 1, :])
                        prb = ps_rb.tile([DK, 512], F32, name="prb")
                        nc.tensor.matmul(prb[:], ones64r[:], r[:], start=True, stop=True)
                        au = small.tile([DK, 512], F32, name="au")
                        nc.vector.tensor_copy(out=au[:], in_=ppv[0:DK, :])
                        nc.vector.tensor_mul(
                            out=AT[po : po + DK, ot, :], in0=au[:], in1=prb[:]
                        )

                    # ---- phase 3: output projection ----
                    for qt in range(NT_Q):
                        for oc in range(2):
                            py = ps_pv.tile([128, 512], F32, name="ppv")
                            for ct in range(NT_D):
                                nc.tensor.matmul(
                                    py[:],
                                    AT[:, ct, qt * 128 : (qt + 1) * 128],
                                    WoT[:, ct, oc * 512 : (oc + 1) * 512],
                                    start=(ct == 0),
                                    stop=False,
                                )
                            nc.tensor.matmul(
                                py[:],
                                ones1[:],
                                bor[:, oc * 512 : (oc + 1) * 512],
                                start=False,
                                stop=True,
                            )
                            ys = small.tile([128, 512], F32, name="ys")
                            nc.vector.tensor_copy(out=ys[:], in_=py[:])
                            nc.sync.dma_start(
                                out=Y_d[
                                    qt * 128 : (qt + 1) * 128,
                                    oc * 512 : (oc + 1) * 512,
                                ],
                                in_=ys[:],
                            )

            if reps > 1:
                with tc.For_i(0, reps, 1):
                    phase1()
                    phase23()
            else:
                phase1()
                phase23()

    _split_sync_waits(nc)
    return nc


# ---------------------------------------------------------------------------
# Host-side sharding / unsharding
# ---------------------------------------------------------------------------


def prep_core_inputs(x, Wq, bq, Wk, bk, Wv, bv, Wo, bo):
    bf = ml_dtypes.bfloat16
    x = np.asarray(x, dtype=np.float32)
    WqT = np.ascontiguousarray(np.asarray(Wq).T).astype(bf)
    WkT = np.ascontiguousarray(np.asarray(Wk).T).astype(bf)
    WvT = np.ascontiguousarray(np.asarray(Wv).T).astype(bf)
    WoT = np.ascontiguousarray(np.asarray(Wo).T).astype(bf)
    bqt = np.ascontiguousarray(np.asarray(bq, dtype=np.float32).reshape(NT_D, 128).T)
    bkt = np.ascontiguousarray(np.asarray(bk, dtype=np.float32).reshape(NT_D, 128).T)
    bvr = np.asarray(bv).reshape(1, D).astype(bf)
    bor = np.asarray(bo).reshape(1, D).astype(bf)
    in_maps = []
    for c in range(N_CORES):
        b, qc = c // 4, c % 4
        xb = x[b]
        xT = np.ascontiguousarray(xb.T).astype(bf)
        xTq = np.ascontiguousarray(xb[qc * SQ : (qc + 1) * SQ].T).astype(bf)
        in_maps.append(
            dict(
                xT=xT, xTq=xTq, WqT=WqT, WkT=WkT, WvT=WvT, WoT=WoT,
                bqt=bqt, bkt=bkt, bvr=bvr, bor=bor,
            )
        )
    return in_maps


def assemble_output(outs):
    y = np.empty((2, S, D), dtype=np.float32)
    for c in range(N_CORES):
        b, qc = c // 4, c % 4
        y[b, qc * SQ : (qc + 1) * SQ, :] = outs[c]["y"]
    return y


_NC_CACHE = {}


def kernel(**inputs) -> np.ndarray:
    from concourse.bass_utils import run_bass_kernel_spmd

    if "nc" not in _NC_CACHE:
        _NC_CACHE["nc"] = build_mha()
    nc = _NC_CACHE["nc"]
    in_maps = prep_core_inputs(**inputs)
    res = run_bass_kernel_spmd(nc, in_maps, core_ids=list(range(N_CORES)))
    return assemble_output(res.results)
